# revision 3
# baseline (speedup 1.0000x reference)
"""Trainium2 Bass kernel v2 for per-row contiguous segment-mean (ChordModel).

out[b, t, :] = mean over the chord block (contiguous run) containing t.

Design (2-pass PE, chunk-local + static boundary merge):
  - Data parallel over batch: 8 cores x 4 rows; per row 16 chunks of 128.
  - Host precomputes (from the tiny int32 chord_changes input) per-position
    segment "slots" and per-slot inverse counts, in fp16:
      * slot(p) = within-chunk segment id, EXCEPT the last segment of each
        chunk is relocated to slot 127.  This pins the two segments that can
        cross a chunk boundary (head -> slot 0, tail -> slot 127) to STATIC
        partition rows, so the cross-chunk merge needs no data-dependent
        indexing on device.
      * invc_slot[j, c] = 1 / (full merged count of the block at slot j).
  - Pass 1 (compress): P_c = A_c^T @ H_c per chunk, where A_c[p, j] =
    one-hot(slot(p) == j), built in one batched DVE compare per row.
    P holds per-slot partial sums.
  - Boundary merge: blocks crossing a chunk boundary are split between
    slot 127 of chunk c and slot 0 of chunk c+1.  Gather those two rows of
    P into [15, 512] tiles via SBUF->SBUF DMA (the DMA also applies the
    +-1 chunk shift, keeping all DVE ops partition-aligned), cross-add with
    the merge mask, scatter back.  Runs <= 128 never span 3 chunks, so one
    merge step suffices (actual max run here is ~25).
  - Pass 2 (decompress): out_c = A'_c @ P_c with A'[j, t] =
    one-hot(slot(t) == j) * invc_slot[j], built in ONE fused DVE
    scalar_tensor_tensor per row.  Scaling by 1/count rides the stationary,
    so no per-chunk means scaling op exists.
  - All matmuls fp16 (1 cycle/row on PE); PSUM accumulates fp32.  PSUM
    results are copied out in 2-chunk pairs ([128, 2x512] per op) split
    ~11:5 between Act and DVE (DVE also owns the builds + merge), casting
    to fp16.  I/O is fp16 end-to-end (host casts f32->fp16 on entry,
    fp16->f32 on exit), halving HBM traffic.
  - The row loop is software-pipelined by one row (decompress of row r-1 is
    emitted after compress of row r) so the PE never waits on the merge.
"""

import numpy as np

import concourse.bass as bass
import concourse.bacc as bacc
import concourse.mybir as mybir
import concourse.tile as tile

P = 128          # partitions / chunk size
T = 2048         # sequence length
D = 512          # hidden dim
R = 4            # batch rows per core
C = T // P       # chunks per row (16)
G = R * C        # chunks per core (64)
N_CORES = 8

FP32 = mybir.dt.float32
FP16 = mybir.dt.float16
Alu = mybir.AluOpType


def build_body(tc, out_ap, hs_ap, slot_bcast_ap, slotT_ap, invc_slotT_ap,
               msh_ap, repeat=1, lvl=0):
    """Per-core program.

    out/hs:     [R, T, D] fp16 DRAM.
    slot_bcast: [P, G, P] fp16  (slot id per position, replicated over
                                 partitions: [j, g, t] = slot(t in chunk g))
    slotT:      [P, G] fp16     (slot per position, column form)
    invc_slotT: [P, G] fp16     (1/merged count; partition j = slot)
    msh:        [C, R] f32      (msh[c, r] = 1 if chunk c+1 of row r merges
                                 backward into chunk c)
    """
    nc = tc.nc
    # probe levels: each level >0 strips one more dataflow suffix component.
    do_store = lvl < 1
    do_outcopy = lvl < 2
    do_dec = lvl < 3
    do_merge = lvl < 4
    do_pcopy = lvl < 5
    do_compress = lvl < 6
    do_build = lvl < 7
    do_load = lvl < 8

    const = tc.alloc_tile_pool(name="const", bufs=1)
    side = tc.alloc_tile_pool(name="side", bufs=1)

    # iota_f16[p, j] = j (free index); iota_col[p, 0] = p (partition index)
    iota_i = const.tile([P, P], mybir.dt.int32, tag="iota_i")
    nc.gpsimd.iota(iota_i[:], pattern=[[1, P]], base=0, channel_multiplier=0)
    iota_f16 = const.tile([P, P], FP16, tag="iota_f16")
    nc.vector.tensor_copy(iota_f16[:], iota_i[:])
    iota_ci = const.tile([P, 1], mybir.dt.int32, tag="iota_ci")
    nc.gpsimd.iota(iota_ci[:], pattern=[[0, 1]], base=0, channel_multiplier=1)
    iota_col = const.tile([P, 1], FP32, tag="iota_col")
    nc.vector.tensor_copy(iota_col[:], iota_ci[:])

    # aux loads (once per program)
    slotT = side.tile([P, G], FP16, tag="slotT")
    nc.sync.dma_start(out=slotT[:], in_=slotT_ap)
    invc_slotT = side.tile([P, G], FP16, tag="invc_slotT")
    nc.sync.dma_start(out=invc_slotT[:], in_=invc_slotT_ap)
    msh = side.tile([C, R], FP32, tag="msh")
    nc.sync.dma_start(out=msh[:], in_=msh_ap)

    # ---- main loop pools ----
    h_pool = tc.alloc_tile_pool(name="h", bufs=2)
    sb_pool = tc.alloc_tile_pool(name="sbc", bufs=2)      # slot broadcast
    a_pool = tc.alloc_tile_pool(name="amat", bufs=3)
    p_pool = tc.alloc_tile_pool(name="psb", bufs=3)       # per-slot sums
    o_pool = tc.alloc_tile_pool(name="osb", bufs=4)       # out staging
    mg_pool = tc.alloc_tile_pool(name="merge", bufs=3)
    pps_pool = tc.alloc_tile_pool(name="pps", bufs=2, space="PSUM")   # 2x2 bank
    ops_pool = tc.alloc_tile_pool(name="ops", bufs=2, space="PSUM")   # 2x2 bank
    pools = [h_pool, sb_pool, a_pool, p_pool, o_pool, mg_pool,
             pps_pool, ops_pool]

    NPAIR = C // 2       # matmul/copy pairs per row (8)
    OHALF = C // 2       # chunks per output staging tile (8)

    def emit_decompress(r, a_dec, p_sb, na):
        # na = number of the row's 16 paired copies already given to Act
        o_sb = None
        for q in range(NPAIR):
            o_ps = ops_pool.tile([P, 2, D], FP32, tag="o_ps")
            if do_dec:
                for k in range(2):
                    c = 2 * q + k
                    nc.tensor.matmul(o_ps[:, k, :], a_dec[:, c, :],
                                     p_sb[:, c, :], start=True, stop=True)
            if q % (OHALF // 2) == 0:
                o_sb = o_pool.tile([P, OHALF, D], FP16, tag="o_sb")
            oc = (2 * q) % OHALF
            if do_outcopy and do_dec:
                if na + q < 11:
                    nc.scalar.copy(o_sb[:, oc:oc + 2, :], o_ps[:])
                else:
                    nc.vector.tensor_copy(o_sb[:, oc:oc + 2, :], o_ps[:])
            if oc == OHALF - 2 and do_store and do_outcopy and do_dec:
                h0 = (2 * q + 2) - OHALF
                nc.sync.dma_start(
                    out=out_ap[r, h0 * P:(h0 + OHALF) * P, :].rearrange(
                        "(c p) d -> p c d", p=P),
                    in_=o_sb[:])

    pending = []         # deferred decompress, 2-row skew
    rows = [r for _ in range(repeat) for r in range(R)]

    for r in rows:
        cr = slice(r * C, (r + 1) * C)

        # --- loads for row r ---
        h_row = h_pool.tile([P, C, D], FP16, tag="h_row")
        if do_load:
            QC = 4
            for q in range(C // QC):
                nc.sync.dma_start(
                    out=h_row[:, q * QC:(q + 1) * QC, :],
                    in_=hs_ap[r, q * QC * P:(q + 1) * QC * P, :].rearrange(
                        "(c p) d -> p c d", p=P))
        slot_b = sb_pool.tile([P, C, P], mybir.dt.int8, tag="slot_b")
        if do_load:
            nc.sync.dma_start(out=slot_b[:], in_=slot_bcast_ap[:, cr, :])

        # --- stationary builds for row r (DVE) ---
        # compress: A_self[p, c, j] = (slot(p) == j)       [one-hot, raw]
        a_self = a_pool.tile([P, C, P], FP16, tag="a_self")
        a_dec = a_pool.tile([P, C, P], FP16, tag="a_dec")
        if do_build and do_load:
            nc.vector.tensor_tensor(
                a_self[:],
                iota_f16[:].unsqueeze(1).broadcast_to([P, C, P]),
                slotT[:, cr].unsqueeze(2).broadcast_to([P, C, P]),
                Alu.is_equal)
            # decompress: A_dec[j, c, t] = (slot(t) == j) * invc_slot[j, c]
            nc.vector.scalar_tensor_tensor(
                a_dec[:], slot_b[:], iota_col[:],
                invc_slotT[:, cr].unsqueeze(2).broadcast_to([P, C, P]),
                Alu.is_equal, Alu.mult)

        # --- pass 1: compress row r (per-slot partial sums) ---
        p_sb = p_pool.tile([P, C, D], FP16, tag="p_sb")
        na = 0
        for q in range(NPAIR):
            p_ps = pps_pool.tile([P, 2, D], FP32, tag="p_ps")
            if do_compress and do_build and do_load:
                for k in range(2):
                    c = 2 * q + k
                    nc.tensor.matmul(p_ps[:, k, :], a_self[:, c, :],
                                     h_row[:, c, :], start=True, stop=True)
            if do_pcopy and do_compress and do_build and do_load:
                if q < 6:
                    nc.scalar.copy(p_sb[:, 2 * q:2 * q + 2, :], p_ps[:])
                    na += 1
                else:
                    nc.vector.tensor_copy(p_sb[:, 2 * q:2 * q + 2, :], p_ps[:])

        # --- pass 2, two rows back (keeps PE busy during merges) ---
        if len(pending) >= 2:
            emit_decompress(*pending.pop(0))

        # --- boundary merge for row r (DMA + DVE only) ---
        if do_merge and do_pcopy and do_compress and do_build and do_load:
            # heads_sh[c] = P[0, c+1]; tails[c] = P[127, c]   (c = 0..14)
            heads_sh = mg_pool.tile([C - 1, D], FP16, tag="heads_sh")
            nc.sync.dma_start(out=heads_sh[:], in_=p_sb[0:1, 1:C, :])
            tails = mg_pool.tile([C - 1, D], FP16, tag="tails")
            nc.sync.dma_start(out=tails[:], in_=p_sb[P - 1:P, 0:C - 1, :])
            mcol = msh[0:C - 1, r:r + 1]
            tmp1 = mg_pool.tile([C - 1, D], FP16, tag="tmp1")
            nc.vector.tensor_scalar(tmp1[:], heads_sh[:], mcol, None, Alu.mult)
            tmp2 = mg_pool.tile([C - 1, D], FP16, tag="tmp2")
            nc.vector.tensor_scalar(tmp2[:], tails[:], mcol, None, Alu.mult)
            nc.vector.tensor_tensor(tails[:], tails[:], tmp1[:], Alu.add)
            nc.vector.tensor_tensor(heads_sh[:], heads_sh[:], tmp2[:], Alu.add)
            nc.sync.dma_start(out=p_sb[P - 1:P, 0:C - 1, :], in_=tails[:])
            nc.sync.dma_start(out=p_sb[0:1, 1:C, :], in_=heads_sh[:])

        pending.append((r, a_dec, p_sb, na))

    for args in pending:
        emit_decompress(*args)

    for p in reversed(pools):
        p.release()
    side.release()
    const.release()


_CACHE = {}


def _build_program(repeat=1, lvl=0):
    key = ("nc8", repeat, lvl)
    if key in _CACHE:
        return _CACHE[key]
    nc = bacc.Bacc("TRN2", target_bir_lowering=False, debug=False)
    hs = nc.dram_tensor("hs16", [R, T, D], FP16, kind="ExternalInput")
    slot_bcast = nc.dram_tensor("slot_bcast", [P, G, P], mybir.dt.int8,
                                kind="ExternalInput")
    slotT = nc.dram_tensor("slotT", [P, G], FP16, kind="ExternalInput")
    invc_slotT = nc.dram_tensor("invc_slotT", [P, G], FP16,
                                kind="ExternalInput")
    msh = nc.dram_tensor("msh", [C, R], FP32, kind="ExternalInput")
    out = nc.dram_tensor("out16", [R, T, D], FP16, kind="ExternalOutput")
    with tile.TileContext(nc) as tc:
        build_body(tc, out.ap(), hs.ap(), slot_bcast.ap(), slotT.ap(),
                   invc_slotT.ap(), msh.ap(), repeat=repeat, lvl=lvl)
    nc.compile()
    _CACHE[key] = nc
    return nc


def make_in_maps(hidden_states: np.ndarray, chord_changes: np.ndarray):
    """Host prep: fp16 cast + slot/count index tensors, per core."""
    hs16 = np.ascontiguousarray(hidden_states, dtype=np.float16)
    cc = np.ascontiguousarray(chord_changes, dtype=np.int64)
    B = hs16.shape[0]
    assert B == N_CORES * R and hs16.shape[1:] == (T, D)

    ids = np.cumsum(cc, axis=1) - cc[:, :1]          # [B, T] block ids
    ids3 = ids.reshape(B, C, P)
    rel = ids3 - ids3[:, :, :1]                      # within-chunk id
    L = rel[:, :, -1:]                               # last (tail) id
    slot = np.where(rel == L, P - 1, rel).astype(np.int32)   # [B, C, P]

    # full (merged) count per position, from global ids
    nseg = int(ids.max()) + 1
    counts = np.zeros((B, nseg + 1), np.int64)
    for b in range(B):
        counts[b, :] = np.bincount(ids[b], minlength=nseg + 1)
    count_pos = np.take_along_axis(counts, ids, axis=1)      # [B, T]
    invc_pos = (1.0 / np.maximum(count_pos, 1)).astype(np.float16)

    # invc by (chunk, slot): scatter per-position values
    invc_slot = np.ones((B, C, P), np.float16)
    bidx, cidx, _ = np.meshgrid(np.arange(B), np.arange(C), np.arange(P),
                                indexing="ij")
    invc_slot[bidx, cidx, slot] = invc_pos.reshape(B, C, P)

    # merge flag, shifted: msh[b, c] = 1 if chunk c+1 continues chunk c
    msh = np.zeros((B, C), np.float32)
    msh[:, :C - 1] = (cc.reshape(B, C, P)[:, 1:, 0] == 0).astype(np.float32)

    slot16 = slot.astype(np.float16)
    in_maps = []
    for i in range(N_CORES):
        rows = slice(i * R, (i + 1) * R)
        srows = slot16[rows].reshape(G, P)
        in_maps.append({
            "hs16": hs16[rows],
            "slot_bcast": np.ascontiguousarray(np.broadcast_to(
                srows[None, :, :], (P, G, P))).astype(np.int8),
            "slotT": np.ascontiguousarray(srows.T),
            "invc_slotT": np.ascontiguousarray(
                invc_slot[rows].reshape(G, P).T),
            "msh": np.ascontiguousarray(msh[rows].T),   # [C, R]
        })
    return in_maps


def kernel(hidden_states: np.ndarray, chord_changes: np.ndarray) -> np.ndarray:
    from concourse.bass_utils import run_bass_kernel_spmd

    in_maps = make_in_maps(hidden_states, chord_changes)
    nc = _build_program()
    res = run_bass_kernel_spmd(nc, in_maps, list(range(N_CORES)))
    out = np.concatenate([res.results[i]["out16"] for i in range(N_CORES)],
                         axis=0)
    return out.astype(np.float32)
